# revision 4
# baseline (speedup 1.0000x reference)
"""GCN message-passing kernel for 8 Trainium2 NeuronCores (Bass/Tile).

Computation:  out = (segment_sum(relu(x@W1+b1)[edge_src], edge_dst)) @ W2 + b2

v7 "lane=dst / W2-as-scatter" design.

Destination nodes are partitioned across the 8 cores (degree-balanced
serpentine).  Within a core, nodes are degree-sorted into 100 blocks of 128;
4 blocks form a quad (25 quads).  The HOST lays x out per MESSAGE (duplicated
rows, fp8-e3m4, transposed to [feat, msg]) such that a message to destination
offset `lane` of block j sits at column j*128+lane of some 512-wide tile row.
With that layout:

  * message compute is  ph[of, 512msgs] = W1s^T @ xm_tile  with W1 STATIONARY
    (no per-matmul LDWEIGHTS) and 512-wide moving operand;
  * relu lands tiles in an fp8-e4m3 arena (DVE rows are mean-centered via a
    dual-op tensor_scalar, ScalarE rows plain relu; exact host correction);
  * the segment-sum IS the W2 matmul: pa[of2, 512dstlanes] += W2d^T @ ar_pair
    accumulated over tile rows in PSUM, with fp8 DoubleRow contracting TWO
    arena tiles per instruction (K=256).  No one-hot, no transpose, no
    gather - the irregular access lives entirely in host-side indexing.

Scales (exactly compensated on host): x unscaled e3m4, W1*16 e3m4, arena
e4m3 (DVE rows store relu-6.5), W2*8 e4m3 in both DoubleRow planes; device
output is 128*(agg@W2) with known per-quad offsets.
"""

import os
import sys

sys.path.insert(0, "/opt/trn_rl_repo")

import numpy as np

import bass_rust
import concourse.bass as bass
import concourse.bacc as bacc
import concourse.mybir as mybir
import concourse.tile as tile_mod
from concourse.tile import TileContext
from concourse.bass_utils import run_bass_kernel_spmd

NCORES = 8
D = 128
P = 128
QB = 4                      # blocks per quad (512 output columns)
QW = QB * P                 # 512
CKONST = 6.5                # arena centering constant (e4m3-exact), in 16*h units
USE_DR = bool(int(os.environ.get("GCN_USE_DR", "1")))       # fp8 DoubleRow scatter
CENTER = bool(int(os.environ.get("GCN_CENTER", "1")))       # center DVE arena rows

_PATCHED = False


def _patch_tile_drain():
    """This walrus build only accepts ONE sync-wait on a CTRL (Drain)
    instruction; Tile's end-of-kernel drain carries one wait per DMA sem
    lane.  Split the waits across multiple drain instructions."""
    global _PATCHED
    if _PATCHED:
        return
    _PATCHED = True

    def _patched_dab(self, tick_clock, wait_clock):
        nc = self.nc
        from concourse.vector_clock import ScopedClock

        drain_inst = nc.sync.drain()
        wait_clock.add_sem_waits(
            drain_inst.ins, ScopedClock({None: tick_clock.global_clock})
        )
        si = drain_inst.ins.sync_info
        if si is not None and si.on_wait is not None and len(si.on_wait) > 1:
            waits = list(si.on_wait)
            drain_inst.ins.sync_info = bass_rust.SyncInfo(
                on_wait=[waits[0]], on_update=list(si.on_update or [])
            )
            for w in waits[1:]:
                extra = nc.sync.drain()
                extra.ins.sync_info = bass_rust.SyncInfo(on_wait=[w], on_update=[])
        nc.all_engine_barrier()
        assert self.sems is not None
        popped = nc._tile_sem_poison_stack.pop()
        assert popped is self._sem_poison
        nc.clear_and_free_semaphores(list(self.sems.allocated().values()))
        nc.all_engine_barrier()

    tile_mod.TileContext._drain_and_barrier = _patched_dab


def _assign_nodes(deg, n_parts, part_capacity):
    """Degree-balanced serpentine partition of nodes into n_parts."""
    order = np.argsort(-deg, kind="stable")
    part = np.empty(len(deg), np.int32)
    n = len(deg)
    fwd = np.arange(n_parts)
    rev = fwd[::-1]
    pos = 0
    row = 0
    while pos < n:
        chunk = order[pos : pos + n_parts]
        lane = fwd if (row % 2 == 0) else rev
        part[chunk] = lane[: len(chunk)]
        pos += n_parts
        row += 1
    counts = np.bincount(part, minlength=n_parts)
    assert counts.max() <= part_capacity, (counts.max(), part_capacity)
    return part


def _build_host_plan(x, edge_src, edge_dst):
    import ml_dtypes

    N, Dd = x.shape
    E = edge_src.shape[0]
    assert Dd == D
    npc = (N + NCORES - 1) // NCORES          # nodes per core
    nblk = ((npc + P - 1) // P + QB - 1) // QB * QB   # blocks, quad-aligned
    nquad = nblk // QB

    deg = np.bincount(edge_dst, minlength=N).astype(np.int64)
    core_of = _assign_nodes(deg, NCORES, npc)

    # per-core degree-desc ordering -> block / lane of every node
    blk_of = np.empty(N, np.int32)
    off_of = np.empty(N, np.int32)
    NT_b = np.zeros((NCORES, nblk), np.int64)
    for c in range(NCORES):
        nodes_c = np.nonzero(core_of == c)[0]
        o = nodes_c[np.argsort(-deg[nodes_c], kind="stable")]
        idx = np.arange(len(o))
        blk_of[o] = idx // P
        off_of[o] = idx % P
        if len(o):
            bmax = np.zeros(nblk, np.int64)
            np.maximum.at(bmax, idx // P, deg[o])
            NT_b[c] = bmax

    NTq_c = NT_b.reshape(NCORES, nquad, QB).max(axis=2)      # per-core quad NT
    NTq = NTq_c.max(axis=0)                                   # shared template
    NTq = np.maximum(NTq + (NTq % 2), 2)                      # even, >=2
    qstart = np.zeros(nquad + 1, np.int64)
    np.cumsum(NTq, out=qstart[1:])
    ntiles = int(qstart[-1])
    TOTCOL = ntiles * QW

    # per-edge slot
    d = edge_dst
    e_core = core_of[d]
    e_q = blk_of[d] // QB
    e_j = blk_of[d] % QB
    e_lane = off_of[d]
    order = np.argsort(d, kind="stable")
    starts = np.zeros(N + 1, np.int64)
    np.cumsum(np.bincount(d, minlength=N), out=starts[1:])
    rank = np.empty(E, np.int64)
    rank[order] = np.arange(E, dtype=np.int64) - starts[d[order]]
    assert np.all(rank < NTq[e_q]), "rank exceeded quad template"
    e_col = (qstart[e_q] + rank) * QW + e_j * P + e_lane

    # xm: per-core transposed per-message x in fp8 e3m4
    x8 = np.asarray(x, np.float32).astype(ml_dtypes.float8_e3m4)
    xm = np.zeros((NCORES, P, TOTCOL), ml_dtypes.float8_e3m4)
    for c in range(NCORES):
        m = e_core == c
        xm[c][:, e_col[m]] = x8[edge_src[m]].T

    plan = dict(
        N=N, E=E, nblk=nblk, nquad=nquad, npc=npc,
        NTq=NTq, qstart=qstart, ntiles=ntiles, TOTCOL=TOTCOL,
        xm=xm, deg=deg, core_of=core_of, blk_of=blk_of, off_of=off_of,
    )
    return plan


def _build_program(plan, has_b1):
    _patch_tile_drain()
    nquad = plan["nquad"]
    NTq = plan["NTq"]
    qstart = plan["qstart"]
    TOTCOL = plan["TOTCOL"]
    NTmax = int(NTq.max())

    nc = bacc.Bacc("TRN2", debug=False)
    f32 = mybir.dt.float32
    bf16 = mybir.dt.bfloat16
    f8e3 = mybir.dt.float8e3
    f8e4 = mybir.dt.float8e4

    xm_t = nc.dram_tensor("xm", [P, TOTCOL], f8e3, kind="ExternalInput")
    w1_t = nc.dram_tensor("w1", [P, P], f8e3, kind="ExternalInput")
    w2_t = nc.dram_tensor("w2", [P, 2 * P], f8e4, kind="ExternalInput")
    b1_t = nc.dram_tensor("b1", [1, P], bf16, kind="ExternalInput")
    ones_t = nc.dram_tensor("ones", [1, QW], bf16, kind="ExternalInput")
    out_t = nc.dram_tensor("out", [nquad, P, QW], f32, kind="ExternalOutput")

    relu = mybir.ActivationFunctionType.Relu
    op_max = mybir.AluOpType.max
    op_sub = mybir.AluOpType.subtract

    with TileContext(nc) as tc:
        with (
            tc.tile_pool(name="const", bufs=1) as constp,
            tc.tile_pool(name="xm", bufs=3) as xmp,
            tc.tile_pool(name="p1", bufs=6, space="PSUM") as p1,
            tc.tile_pool(name="arena", bufs=3) as arenap,
            tc.tile_pool(name="p2", bufs=2, space="PSUM") as p2,
            tc.tile_pool(name="outp", bufs=4) as outp,
        ):
            w1s = constp.tile([P, P], f8e3, tag="w1")
            nc.sync.dma_start(w1s[:], w1_t[:])
            w2s = constp.tile([P, 2 * P], f8e4, tag="w2")
            nc.sync.dma_start(w2s[:], w2_t[:])
            b1s = constp.tile([1, P], bf16, tag="b1")
            nc.sync.dma_start(b1s[:], b1_t[:])
            oness = constp.tile([1, QW], bf16, tag="ones")
            nc.sync.dma_start(oness[:], ones_t[:])
            w2v = w2s[:].rearrange("p (two m) -> p two m", two=2)

            def emit_scatter(q, ar):
                """Segment-sum + W2 for quad q (arena rows already relu'd)."""
                NT = int(NTq[q])
                pa = p2.tile([P, QW], f32, tag="p2")
                if USE_DR:
                    npair = NT // 2
                    for tp in range(npair):
                        nc.tensor.matmul(
                            pa[:], w2v,
                            ar[:, 2 * tp : 2 * tp + 2, :],
                            start=(tp == 0), stop=(tp == npair - 1),
                            perf_mode=mybir.MatmulPerfMode.DoubleRow,
                        )
                else:
                    for t in range(NT):
                        nc.tensor.matmul(
                            pa[:], w2v[:, 0, :], ar[:, t, :],
                            start=(t == 0), stop=(t == NT - 1),
                        )
                ot = outp.tile([P, QW], f32, tag="ot")
                if q % 2 == 0:
                    nc.scalar.activation(
                        ot[:], pa[:], mybir.ActivationFunctionType.Copy
                    )
                else:
                    nc.vector.tensor_copy(ot[:], pa[:])
                nc.sync.dma_start(out_t[q], ot[:])

            pending = None       # (q, arena) whose scatter is delayed one quad
            for q in range(nquad):
                NT = int(NTq[q])
                qs = int(qstart[q])
                xm = xmp.tile([P, NTmax * QW], f8e3, tag="xm")
                nc.sync.dma_start(
                    xm[:, : NT * QW], xm_t[:, qs * QW : (qs + NT) * QW]
                )
                ar = arenap.tile([P, NTmax, QW], f8e4, tag="ar")
                for t in range(NT):
                    ph = p1.tile([P, QW], f32, tag="p1")
                    if has_b1:
                        nc.tensor.matmul(
                            ph[:], w1s[:], xm[:, t * QW : (t + 1) * QW],
                            start=True, stop=False,
                        )
                        nc.tensor.matmul(
                            ph[:], b1s[:], oness[:], start=False, stop=True
                        )
                    else:
                        nc.tensor.matmul(
                            ph[:], w1s[:], xm[:, t * QW : (t + 1) * QW],
                            start=True, stop=True,
                        )
                    if CENTER and t % 2 == 0:
                        nc.vector.tensor_scalar(
                            ar[:, t, :], ph[:], 0.0, CKONST, op_max, op_sub
                        )
                    elif t % 2 == 1:
                        nc.scalar.activation(ar[:, t, :], ph[:], relu)
                    else:
                        nc.vector.tensor_scalar(
                            ar[:, t, :], ph[:], 0.0, None, op_max
                        )
                    # interleave the delayed scatter: one pair per two rows
                    if pending is not None and t % 2 == 1:
                        pq, par = pending
                        pNT = int(NTq[pq])
                        tp = t // 2
                        if USE_DR and tp < pNT // 2:
                            nc.tensor.matmul(
                                pa_pend[:], w2v,
                                par[:, 2 * tp : 2 * tp + 2, :],
                                start=(tp == 0), stop=(tp == pNT // 2 - 1),
                                perf_mode=mybir.MatmulPerfMode.DoubleRow,
                            )
                if pending is not None:
                    pq, par = pending
                    pNT = int(NTq[pq])
                    if USE_DR:
                        # finish any pairs not covered by the interleave
                        done = min(NT // 2, pNT // 2)
                        for tp in range(done, pNT // 2):
                            nc.tensor.matmul(
                                pa_pend[:], w2v,
                                par[:, 2 * tp : 2 * tp + 2, :],
                                start=(tp == 0), stop=(tp == pNT // 2 - 1),
                                perf_mode=mybir.MatmulPerfMode.DoubleRow,
                            )
                    else:
                        for t in range(pNT):
                            nc.tensor.matmul(
                                pa_pend[:], w2v[:, 0, :], par[:, t, :],
                                start=(t == 0), stop=(t == pNT - 1),
                            )
                    ot = outp.tile([P, QW], f32, tag="ot")
                    if pq % 2 == 0:
                        nc.scalar.activation(
                            ot[:], pa_pend[:], mybir.ActivationFunctionType.Copy
                        )
                    else:
                        nc.vector.tensor_copy(ot[:], pa_pend[:])
                    nc.sync.dma_start(out_t[pq], ot[:])
                pending = (q, ar)
                pa_pend = p2.tile([P, QW], f32, tag="p2")
            if pending is not None:
                emit_scatter(pending[0], pending[1])

    nc.compile()
    return nc


def kernel(x, edge_src, edge_dst, W1, b1, W2, b2, _trace=False, _ret_stats=False):
    import ml_dtypes

    x = np.asarray(x, np.float32)
    edge_src = np.asarray(edge_src).astype(np.int64)
    edge_dst = np.asarray(edge_dst).astype(np.int64)
    W1 = np.asarray(W1, np.float32).reshape(D, D)
    W2 = np.asarray(W2, np.float32).reshape(D, D)
    b1 = np.asarray(b1, np.float32).reshape(D)
    b2 = np.asarray(b2, np.float32).reshape(D)
    has_b1 = bool(np.any(b1))

    plan = _build_host_plan(x, edge_src, edge_dst)

    w1s = (16.0 * W1).astype(ml_dtypes.float8_e3m4)
    w2sc = (8.0 * W2).astype(ml_dtypes.float8_e4m3)
    w2d = np.concatenate([w2sc, w2sc], axis=1)        # [128, 2*128] both planes
    b1s = (16.0 * b1).reshape(1, P).astype(ml_dtypes.bfloat16)
    ones = np.ones((1, QW), ml_dtypes.bfloat16)

    in_maps = []
    for c in range(NCORES):
        in_maps.append({
            "xm": plan["xm"][c],
            "w1": w1s,
            "w2": w2d,
            "b1": b1s,
            "ones": ones,
        })

    nc = _build_program(plan, has_b1)
    res = run_bass_kernel_spmd(nc, in_maps, core_ids=list(range(NCORES)), trace=_trace)

    N = plan["N"]
    nquad = plan["nquad"]
    NTq = plan["NTq"]
    deg = plan["deg"]
    core_of = plan["core_of"]
    blk_of = plan["blk_of"]
    off_of = plan["off_of"]

    # exact de-scaling + corrections (see module docstring)
    # device pa[of, col] = sum_msgs (16*h)*(8*W2)  - 6.5*ncent_q*(8*W2).sum
    #                      (+ pads of uncentered rows: relu(16*b1)@8W2 each)
    W2sum = W2.sum(axis=0)                            # [128]
    relu_b1 = np.maximum(16.0 * b1, 0.0)
    padvec = relu_b1 @ W2 / 16.0                      # per-pad-slot pollution/128
    ncent = (NTq // 2) if CENTER else np.zeros(nquad, np.int64)
    # uncentered rows: NT - ncent; pads exist in all NT rows; centered pads
    # store relu(16*b1) - 6.5 (the -6.5 handled by ncent term), uncentered
    # pads store relu(16*b1).  Real-message rows already include b1 legit.

    out = np.zeros((N, D), np.float32)
    nodes = np.arange(N)
    q_of = blk_of[nodes] // QB
    col_of = (blk_of[nodes] % QB) * P + off_of[nodes]
    for c in range(NCORES):
        o = np.asarray(res.results[c]["out"], np.float32)   # [nquad, 128, 512]
        m = core_of == c
        nn = nodes[m]
        out[nn] = o[q_of[nn], :, col_of[nn]] / 128.0
    # corrections (same for every core given shared template)
    out += (CKONST / 16.0) * ncent[q_of][:, None] * W2sum[None, :]
    if has_b1:
        out -= (NTq[q_of] - deg)[:, None] * padvec[None, :]
    if np.any(b2):
        out += b2[None, :]

    if _ret_stats:
        return out, res
    return out


# revision 9
# speedup vs baseline: 1.1678x; 1.1678x over previous
"""GCN message-passing kernel for 8 Trainium2 NeuronCores (Bass/Tile).

Computation:  out = (segment_sum(relu(x@W1+b1)[edge_src], edge_dst)) @ W2 + b2

v7 "lane=dst / W2-as-scatter" design.

Destination nodes are partitioned across the 8 cores (degree-balanced
serpentine).  Within a core, nodes are degree-sorted into 100 blocks of 128;
4 blocks form a quad (25 quads).  The HOST lays x out per MESSAGE (duplicated
rows, fp8-e3m4, transposed to [feat, msg]) such that a message to destination
offset `lane` of block j sits at column j*128+lane of some 512-wide tile row.
With that layout:

  * message compute is  ph[of, 512msgs] = W1s^T @ xm_tile  with W1 STATIONARY
    (no per-matmul LDWEIGHTS) and 512-wide moving operand;
  * relu lands tiles in an fp8-e4m3 arena (DVE rows are mean-centered via a
    dual-op tensor_scalar, ScalarE rows plain relu; exact host correction);
  * the segment-sum IS the W2 matmul: pa[of2, 512dstlanes] += W2d^T @ ar_pair
    accumulated over tile rows in PSUM, with fp8 DoubleRow contracting TWO
    arena tiles per instruction (K=256).  No one-hot, no transpose, no
    gather - the irregular access lives entirely in host-side indexing.

Scales (exactly compensated on host): x unscaled e3m4, W1*16 e3m4, arena
e4m3 (DVE rows store relu-6.5), W2*8 e4m3 in both DoubleRow planes; device
output is 128*(agg@W2) with known per-quad offsets.
"""

import os
import sys

sys.path.insert(0, "/opt/trn_rl_repo")

import numpy as np

import bass_rust
import concourse.bass as bass
import concourse.bacc as bacc
import concourse.mybir as mybir
import concourse.tile as tile_mod
from concourse.tile import TileContext
from concourse.bass_utils import run_bass_kernel_spmd

NCORES = 8
D = 128
P = 128
QB = 4                      # blocks per quad (512 output columns)
QW = QB * P                 # 512
CKONST = 6.5                # arena centering constant (e4m3-exact), in 16*h units
USE_DR = bool(int(os.environ.get("GCN_USE_DR", "1")))       # fp8 DoubleRow scatter
CENTER = bool(int(os.environ.get("GCN_CENTER", "1")))       # center DVE arena rows

_PATCHED = False


def _patch_tile_drain():
    """This walrus build only accepts ONE sync-wait on a CTRL (Drain)
    instruction; Tile's end-of-kernel drain carries one wait per DMA sem
    lane.  Split the waits across multiple drain instructions."""
    global _PATCHED
    if _PATCHED:
        return
    _PATCHED = True

    def _patched_dab(self, tick_clock, wait_clock):
        nc = self.nc
        from concourse.vector_clock import ScopedClock

        drain_inst = nc.sync.drain()
        wait_clock.add_sem_waits(
            drain_inst.ins, ScopedClock({None: tick_clock.global_clock})
        )
        si = drain_inst.ins.sync_info
        if si is not None and si.on_wait is not None and len(si.on_wait) > 1:
            waits = list(si.on_wait)
            drain_inst.ins.sync_info = bass_rust.SyncInfo(
                on_wait=[waits[0]], on_update=list(si.on_update or [])
            )
            for w in waits[1:]:
                extra = nc.sync.drain()
                extra.ins.sync_info = bass_rust.SyncInfo(on_wait=[w], on_update=[])
        nc.all_engine_barrier()
        assert self.sems is not None
        popped = nc._tile_sem_poison_stack.pop()
        assert popped is self._sem_poison
        nc.clear_and_free_semaphores(list(self.sems.allocated().values()))
        nc.all_engine_barrier()

    tile_mod.TileContext._drain_and_barrier = _patched_dab


def _assign_nodes(deg, n_parts, part_capacity):
    """Degree-balanced serpentine partition of nodes into n_parts."""
    order = np.argsort(-deg, kind="stable")
    part = np.empty(len(deg), np.int32)
    n = len(deg)
    fwd = np.arange(n_parts)
    rev = fwd[::-1]
    pos = 0
    row = 0
    while pos < n:
        chunk = order[pos : pos + n_parts]
        lane = fwd if (row % 2 == 0) else rev
        part[chunk] = lane[: len(chunk)]
        pos += n_parts
        row += 1
    counts = np.bincount(part, minlength=n_parts)
    assert counts.max() <= part_capacity, (counts.max(), part_capacity)
    return part


def _build_host_plan(x, edge_src, edge_dst):
    import ml_dtypes

    N, Dd = x.shape
    E = edge_src.shape[0]
    assert Dd == D
    npc = (N + NCORES - 1) // NCORES          # nodes per core
    nblk = ((npc + P - 1) // P + QB - 1) // QB * QB   # blocks, quad-aligned
    nquad = nblk // QB

    deg = np.bincount(edge_dst, minlength=N).astype(np.int64)
    core_of = _assign_nodes(deg, NCORES, npc)

    # per-core degree-desc ordering -> block / lane of every node
    blk_of = np.empty(N, np.int32)
    off_of = np.empty(N, np.int32)
    NT_b = np.zeros((NCORES, nblk), np.int64)
    for c in range(NCORES):
        nodes_c = np.nonzero(core_of == c)[0]
        o = nodes_c[np.argsort(-deg[nodes_c], kind="stable")]
        idx = np.arange(len(o))
        blk_of[o] = idx // P
        off_of[o] = idx % P
        if len(o):
            bmax = np.zeros(nblk, np.int64)
            np.maximum.at(bmax, idx // P, deg[o])
            NT_b[c] = bmax

    NTq_c = NT_b.reshape(NCORES, nquad, QB).max(axis=2)      # per-core quad NT
    NTq = NTq_c.max(axis=0)                                   # shared template
    NTq = np.maximum(NTq + (NTq % 2), 2)                      # even, >=2
    qstart = np.zeros(nquad + 1, np.int64)
    np.cumsum(NTq, out=qstart[1:])
    ntiles = int(qstart[-1])
    TOTCOL = ntiles * QW

    # per-edge slot
    d = edge_dst
    e_core = core_of[d]
    e_q = blk_of[d] // QB
    e_j = blk_of[d] % QB
    e_lane = off_of[d]
    order = np.argsort(d, kind="stable")
    starts = np.zeros(N + 1, np.int64)
    np.cumsum(np.bincount(d, minlength=N), out=starts[1:])
    rank = np.empty(E, np.int64)
    rank[order] = np.arange(E, dtype=np.int64) - starts[d[order]]
    assert np.all(rank < NTq[e_q]), "rank exceeded quad template"
    e_col = (qstart[e_q] + rank) * QW + e_j * P + e_lane

    # xm: per-core transposed per-message x in fp8 e3m4
    x8 = np.asarray(x, np.float32).astype(ml_dtypes.float8_e3m4)
    xm = np.zeros((NCORES, P, TOTCOL), ml_dtypes.float8_e3m4)
    for c in range(NCORES):
        m = e_core == c
        xm[c][:, e_col[m]] = x8[edge_src[m]].T

    plan = dict(
        N=N, E=E, nblk=nblk, nquad=nquad, npc=npc,
        NTq=NTq, qstart=qstart, ntiles=ntiles, TOTCOL=TOTCOL,
        xm=xm, deg=deg, core_of=core_of, blk_of=blk_of, off_of=off_of,
    )
    return plan


def _build_program(plan, has_b1):
    _patch_tile_drain()
    nquad = plan["nquad"]
    NTq = plan["NTq"]
    qstart = plan["qstart"]
    TOTCOL = plan["TOTCOL"]
    NTmax = int(NTq.max())

    nc = bacc.Bacc("TRN2", debug=False)
    f32 = mybir.dt.float32
    f32r = mybir.dt.float32r
    bf16 = mybir.dt.bfloat16
    f8e3 = mybir.dt.float8e3
    f8e4 = mybir.dt.float8e4

    xm_t = nc.dram_tensor("xm", [P, TOTCOL], f8e3, kind="ExternalInput")
    w1_t = nc.dram_tensor("w1", [P, P], f8e3, kind="ExternalInput")
    w2_t = nc.dram_tensor("w2", [P, 2 * P], f8e4, kind="ExternalInput")
    w2f_t = nc.dram_tensor("w2f", [P, P], f32, kind="ExternalInput")
    b1_t = nc.dram_tensor("b1", [1, P], bf16, kind="ExternalInput")
    ones_t = nc.dram_tensor("ones", [1, QW], bf16, kind="ExternalInput")
    out_t = nc.dram_tensor("out", [nquad, P, QW], f32, kind="ExternalOutput")

    relu = mybir.ActivationFunctionType.Relu
    op_max = mybir.AluOpType.max
    op_sub = mybir.AluOpType.subtract

    # pair p (rows 2p, 2p+1) -> ScalarE (arena + fp8 DoubleRow scatter) or
    # VectorE (fused relu+accumulate into an f32 accumulator, f32r scatter).
    SFRAC = float(os.environ.get("GCN_SFRAC", "0.54"))

    def _is_scalar_pair(k):
        return int((k + 1) * SFRAC) > int(k * SFRAC)

    with TileContext(nc) as tc:
        with (
            tc.tile_pool(name="const", bufs=1) as constp,
            tc.tile_pool(name="xm", bufs=3) as xmp,
            tc.tile_pool(name="p1", bufs=3, space="PSUM") as p1,
            tc.tile_pool(name="arena", bufs=2) as arenap,
            tc.tile_pool(name="acc", bufs=3) as accp,
            tc.tile_pool(name="p2", bufs=2, space="PSUM") as p2,
            tc.tile_pool(name="outp", bufs=4) as outp,
        ):
            w1s = constp.tile([P, P], f8e3, tag="w1")
            nc.sync.dma_start(w1s[:], w1_t[:])
            w2s = constp.tile([P, 2 * P], f8e4, tag="w2")
            nc.sync.dma_start(w2s[:], w2_t[:])
            w2f = constp.tile([P, P], f32, tag="w2f")
            nc.sync.dma_start(w2f[:], w2f_t[:])
            w2fc = constp.tile([P, P], f32r, tag="w2fc")
            nc.vector.tensor_copy(w2fc[:], w2f[:])
            b1s = constp.tile([1, P], bf16, tag="b1")
            nc.sync.dma_start(b1s[:], b1_t[:])
            oness = constp.tile([1, QW], bf16, tag="ones")
            nc.sync.dma_start(oness[:], ones_t[:])
            w2v = w2s[:].rearrange("p (two m) -> p two m", two=2)
            w2fr = w2fc[:]

            def emit_scatter(pq, par, pacc, psc):
                """pa = sum over scalar pairs (fp8 DR) + acc halves (f32r)."""
                pa = p2.tile([P, QW], f32, tag="p2")
                total = len(psc) + (2 if pacc is not None else 0)
                k = 0
                for p in psc:
                    nc.tensor.matmul(
                        pa[:], w2v, par[:, 2 * p : 2 * p + 2, :],
                        start=(k == 0), stop=(k == total - 1),
                        perf_mode=mybir.MatmulPerfMode.DoubleRow,
                    )
                    k += 1
                if pacc is not None:
                    for h in range(2):
                        nc.tensor.matmul(
                            pa[:], w2fr,
                            pacc[:, h * QW : (h + 1) * QW],
                            start=(k == 0), stop=(k == total - 1),
                        )
                        k += 1
                ot = outp.tile([P, QW], f32, tag="ot")
                if pq % 2 == 0:
                    nc.scalar.activation(
                        ot[:], pa[:], mybir.ActivationFunctionType.Copy
                    )
                else:
                    nc.vector.tensor_copy(ot[:], pa[:])
                nc.sync.dma_start(out_t[pq], ot[:])

            pending = None
            kpair = 0
            for q in range(nquad):
                NT = int(NTq[q])
                qs = int(qstart[q])
                npair = NT // 2
                xm = xmp.tile([P, NTmax * QW], f8e3, tag="xm")
                nc.sync.dma_start(
                    xm[:, : NT * QW], xm_t[:, qs * QW : (qs + NT) * QW]
                )
                ar = arenap.tile([P, NTmax, QW], f8e4, tag="ar")
                acc = None
                psc = []
                for p in range(npair):
                    ph2 = p1.tile([P, 2 * QW], f32, tag="p1")
                    for h in range(2):
                        t = 2 * p + h
                        nc.tensor.matmul(
                            ph2[:, h * QW : (h + 1) * QW], w1s[:],
                            xm[:, t * QW : (t + 1) * QW],
                            start=True, stop=not has_b1,
                        )
                        if has_b1:
                            nc.tensor.matmul(
                                ph2[:, h * QW : (h + 1) * QW], b1s[:],
                                oness[:], start=False, stop=True,
                            )
                    if _is_scalar_pair(kpair):
                        psc.append(p)
                        ph2v = ph2[:].rearrange("p (two w) -> p two w", two=2)
                        nc.scalar.activation(
                            ar[:, 2 * p : 2 * p + 2, :], ph2v, relu
                        )
                    elif acc is None:
                        acc = accp.tile([P, 2 * QW], f32r, tag="acc")
                        nc.vector.tensor_scalar(
                            acc[:], ph2[:], 0.0, None, op_max
                        )
                    else:
                        nc.vector.scalar_tensor_tensor(
                            acc[:], ph2[:], 0.0, acc[:],
                            op_max, mybir.AluOpType.add,
                        )
                    kpair += 1
                if pending is not None:
                    emit_scatter(*pending)
                pending = (q, ar, acc, psc)
            if pending is not None:
                emit_scatter(*pending)

    nc.compile()
    return nc


def kernel(x, edge_src, edge_dst, W1, b1, W2, b2, _trace=False, _ret_stats=False):
    import ml_dtypes

    x = np.asarray(x, np.float32)
    edge_src = np.asarray(edge_src).astype(np.int64)
    edge_dst = np.asarray(edge_dst).astype(np.int64)
    W1 = np.asarray(W1, np.float32).reshape(D, D)
    W2 = np.asarray(W2, np.float32).reshape(D, D)
    b1 = np.asarray(b1, np.float32).reshape(D)
    b2 = np.asarray(b2, np.float32).reshape(D)
    has_b1 = bool(np.any(b1))

    plan = _build_host_plan(x, edge_src, edge_dst)

    w1s = (16.0 * W1).astype(ml_dtypes.float8_e3m4)
    w2sc = (8.0 * W2).astype(ml_dtypes.float8_e4m3)
    w2d = np.concatenate([w2sc, w2sc], axis=1)        # [128, 2*128] both planes
    w2f = (8.0 * W2).astype(np.float32)
    b1s = (16.0 * b1).reshape(1, P).astype(ml_dtypes.bfloat16)
    ones = np.ones((1, QW), ml_dtypes.bfloat16)

    in_maps = []
    for c in range(NCORES):
        in_maps.append({
            "xm": plan["xm"][c],
            "w1": w1s,
            "w2": w2d,
            "w2f": w2f,
            "b1": b1s,
            "ones": ones,
        })

    nc = _build_program(plan, has_b1)
    res = run_bass_kernel_spmd(nc, in_maps, core_ids=list(range(NCORES)), trace=_trace)

    N = plan["N"]
    nquad = plan["nquad"]
    NTq = plan["NTq"]
    deg = plan["deg"]
    core_of = plan["core_of"]
    blk_of = plan["blk_of"]
    off_of = plan["off_of"]

    # exact de-scaling + corrections (see module docstring)
    # device pa[of, col] = sum_msgs (16*h)*(8*W2); pads contribute
    # relu(16*b1)@(8*W2) each (zero when b1 == 0).
    relu_b1 = np.maximum(16.0 * b1, 0.0)
    padvec = relu_b1 @ W2 / 16.0                      # per-pad-slot pollution/128

    out = np.zeros((N, D), np.float32)
    nodes = np.arange(N)
    q_of = blk_of[nodes] // QB
    col_of = (blk_of[nodes] % QB) * P + off_of[nodes]
    for c in range(NCORES):
        o = np.asarray(res.results[c]["out"], np.float32)   # [nquad, 128, 512]
        m = core_of == c
        nn = nodes[m]
        out[nn] = o[q_of[nn], :, col_of[nn]] / 128.0
    if has_b1:
        out -= (NTq[q_of] - deg)[:, None] * padvec[None, :]
    if np.any(b2):
        out += b2[None, :]

    if _ret_stats:
        return out, res
    return out


# revision 11
# speedup vs baseline: 1.1685x; 1.0006x over previous
"""GCN message-passing kernel for 8 Trainium2 NeuronCores (Bass/Tile).

Computation:  out = (segment_sum(relu(x@W1+b1)[edge_src], edge_dst)) @ W2 + b2

v7 "lane=dst / W2-as-scatter" design.

Destination nodes are partitioned across the 8 cores (degree-balanced
serpentine).  Within a core, nodes are degree-sorted into 100 blocks of 128;
4 blocks form a quad (25 quads).  The HOST lays x out per MESSAGE (duplicated
rows, fp8-e3m4, transposed to [feat, msg]) such that a message to destination
offset `lane` of block j sits at column j*128+lane of some 512-wide tile row.
With that layout:

  * message compute is  ph[of, 512msgs] = W1s^T @ xm_tile  with W1 STATIONARY
    (no per-matmul LDWEIGHTS) and 512-wide moving operand;
  * relu lands tiles in an fp8-e4m3 arena (DVE rows are mean-centered via a
    dual-op tensor_scalar, ScalarE rows plain relu; exact host correction);
  * the segment-sum IS the W2 matmul: pa[of2, 512dstlanes] += W2d^T @ ar_pair
    accumulated over tile rows in PSUM, with fp8 DoubleRow contracting TWO
    arena tiles per instruction (K=256).  No one-hot, no transpose, no
    gather - the irregular access lives entirely in host-side indexing.

Scales (exactly compensated on host): x unscaled e3m4, W1*16 e3m4, arena
e4m3 (DVE rows store relu-6.5), W2*8 e4m3 in both DoubleRow planes; device
output is 128*(agg@W2) with known per-quad offsets.
"""

import os
import sys

sys.path.insert(0, "/opt/trn_rl_repo")

import numpy as np

import bass_rust
import concourse.bass as bass
import concourse.bacc as bacc
import concourse.mybir as mybir
import concourse.tile as tile_mod
from concourse.tile import TileContext
from concourse.bass_utils import run_bass_kernel_spmd

NCORES = 8
D = 128
P = 128
QB = 4                      # blocks per quad (512 output columns)
QW = QB * P                 # 512
CKONST = 6.5                # arena centering constant (e4m3-exact), in 16*h units
USE_DR = bool(int(os.environ.get("GCN_USE_DR", "1")))       # fp8 DoubleRow scatter
CENTER = bool(int(os.environ.get("GCN_CENTER", "1")))       # center DVE arena rows

_PATCHED = False


def _patch_tile_drain():
    """This walrus build only accepts ONE sync-wait on a CTRL (Drain)
    instruction; Tile's end-of-kernel drain carries one wait per DMA sem
    lane.  Split the waits across multiple drain instructions."""
    global _PATCHED
    if _PATCHED:
        return
    _PATCHED = True

    def _patched_dab(self, tick_clock, wait_clock):
        nc = self.nc
        from concourse.vector_clock import ScopedClock

        drain_inst = nc.sync.drain()
        wait_clock.add_sem_waits(
            drain_inst.ins, ScopedClock({None: tick_clock.global_clock})
        )
        si = drain_inst.ins.sync_info
        if si is not None and si.on_wait is not None and len(si.on_wait) > 1:
            waits = list(si.on_wait)
            drain_inst.ins.sync_info = bass_rust.SyncInfo(
                on_wait=[waits[0]], on_update=list(si.on_update or [])
            )
            for w in waits[1:]:
                extra = nc.sync.drain()
                extra.ins.sync_info = bass_rust.SyncInfo(on_wait=[w], on_update=[])
        nc.all_engine_barrier()
        assert self.sems is not None
        popped = nc._tile_sem_poison_stack.pop()
        assert popped is self._sem_poison
        nc.clear_and_free_semaphores(list(self.sems.allocated().values()))
        nc.all_engine_barrier()

    tile_mod.TileContext._drain_and_barrier = _patched_dab


def _assign_nodes(deg, n_parts, part_capacity):
    """Degree-balanced serpentine partition of nodes into n_parts."""
    order = np.argsort(-deg, kind="stable")
    part = np.empty(len(deg), np.int32)
    n = len(deg)
    fwd = np.arange(n_parts)
    rev = fwd[::-1]
    pos = 0
    row = 0
    while pos < n:
        chunk = order[pos : pos + n_parts]
        lane = fwd if (row % 2 == 0) else rev
        part[chunk] = lane[: len(chunk)]
        pos += n_parts
        row += 1
    counts = np.bincount(part, minlength=n_parts)
    assert counts.max() <= part_capacity, (counts.max(), part_capacity)
    return part


def _build_host_plan(x, edge_src, edge_dst):
    import ml_dtypes

    N, Dd = x.shape
    E = edge_src.shape[0]
    assert Dd == D
    npc = (N + NCORES - 1) // NCORES          # nodes per core
    nblk = ((npc + P - 1) // P + QB - 1) // QB * QB   # blocks, quad-aligned
    nquad = nblk // QB

    deg = np.bincount(edge_dst, minlength=N).astype(np.int64)
    core_of = _assign_nodes(deg, NCORES, npc)

    # per-core degree-desc ordering -> block / lane of every node
    blk_of = np.empty(N, np.int32)
    off_of = np.empty(N, np.int32)
    NT_b = np.zeros((NCORES, nblk), np.int64)
    for c in range(NCORES):
        nodes_c = np.nonzero(core_of == c)[0]
        o = nodes_c[np.argsort(-deg[nodes_c], kind="stable")]
        idx = np.arange(len(o))
        blk_of[o] = idx // P
        off_of[o] = idx % P
        if len(o):
            bmax = np.zeros(nblk, np.int64)
            np.maximum.at(bmax, idx // P, deg[o])
            NT_b[c] = bmax

    NTq_c = NT_b.reshape(NCORES, nquad, QB).max(axis=2)      # per-core quad NT
    NTq = NTq_c.max(axis=0)                                   # shared template
    NTq = np.maximum(NTq, 2)                                  # >= 2 (odd ok)
    qstart = np.zeros(nquad + 1, np.int64)
    np.cumsum(NTq, out=qstart[1:])
    ntiles = int(qstart[-1])
    TOTCOL = ntiles * QW

    # per-edge slot
    d = edge_dst
    e_core = core_of[d]
    e_q = blk_of[d] // QB
    e_j = blk_of[d] % QB
    e_lane = off_of[d]
    order = np.argsort(d, kind="stable")
    starts = np.zeros(N + 1, np.int64)
    np.cumsum(np.bincount(d, minlength=N), out=starts[1:])
    rank = np.empty(E, np.int64)
    rank[order] = np.arange(E, dtype=np.int64) - starts[d[order]]
    assert np.all(rank < NTq[e_q]), "rank exceeded quad template"
    e_col = (qstart[e_q] + rank) * QW + e_j * P + e_lane

    # xm: per-core transposed per-message x in fp8 e3m4
    x8 = np.asarray(x, np.float32).astype(ml_dtypes.float8_e3m4)
    xm = np.zeros((NCORES, P, TOTCOL), ml_dtypes.float8_e3m4)
    for c in range(NCORES):
        m = e_core == c
        xm[c][:, e_col[m]] = x8[edge_src[m]].T

    plan = dict(
        N=N, E=E, nblk=nblk, nquad=nquad, npc=npc,
        NTq=NTq, qstart=qstart, ntiles=ntiles, TOTCOL=TOTCOL,
        xm=xm, deg=deg, core_of=core_of, blk_of=blk_of, off_of=off_of,
    )
    return plan


def _build_program(plan, has_b1):
    _patch_tile_drain()
    nquad = plan["nquad"]
    NTq = plan["NTq"]
    qstart = plan["qstart"]
    TOTCOL = plan["TOTCOL"]
    NTmax = int(NTq.max())

    nc = bacc.Bacc("TRN2", debug=False)
    f32 = mybir.dt.float32
    f32r = mybir.dt.float32r
    bf16 = mybir.dt.bfloat16
    f8e3 = mybir.dt.float8e3
    f8e4 = mybir.dt.float8e4

    xm_t = nc.dram_tensor("xm", [P, TOTCOL], f8e3, kind="ExternalInput")
    w1_t = nc.dram_tensor("w1", [P, P], f8e3, kind="ExternalInput")
    w2_t = nc.dram_tensor("w2", [P, 2 * P], f8e4, kind="ExternalInput")
    w2f_t = nc.dram_tensor("w2f", [P, P], f32, kind="ExternalInput")
    b1_t = nc.dram_tensor("b1", [1, P], bf16, kind="ExternalInput")
    ones_t = nc.dram_tensor("ones", [1, QW], bf16, kind="ExternalInput")
    out_t = nc.dram_tensor("out", [nquad, P, QW], f32, kind="ExternalOutput")

    relu = mybir.ActivationFunctionType.Relu
    op_max = mybir.AluOpType.max
    op_sub = mybir.AluOpType.subtract

    # pair p (rows 2p, 2p+1) -> ScalarE (arena + fp8 DoubleRow scatter) or
    # VectorE (fused relu+accumulate into an f32 accumulator, f32r scatter).
    SFRAC = float(os.environ.get("GCN_SFRAC", "0.52"))

    def _is_scalar_pair(k):
        return int((k + 1) * SFRAC) > int(k * SFRAC)

    with TileContext(nc) as tc:
        with (
            tc.tile_pool(name="const", bufs=1) as constp,
            tc.tile_pool(name="xm", bufs=3) as xmp,
            tc.tile_pool(name="p1", bufs=3, space="PSUM") as p1,
            tc.tile_pool(name="arena", bufs=2) as arenap,
            tc.tile_pool(name="acc", bufs=3) as accp,
            tc.tile_pool(name="p2", bufs=2, space="PSUM") as p2,
            tc.tile_pool(name="outp", bufs=4) as outp,
        ):
            w1s = constp.tile([P, P], f8e3, tag="w1")
            nc.sync.dma_start(w1s[:], w1_t[:])
            w2s = constp.tile([P, 2 * P], f8e4, tag="w2")
            nc.sync.dma_start(w2s[:], w2_t[:])
            w2f = constp.tile([P, P], f32, tag="w2f")
            nc.sync.dma_start(w2f[:], w2f_t[:])
            w2fc = constp.tile([P, P], f32r, tag="w2fc")
            nc.vector.tensor_copy(w2fc[:], w2f[:])
            b1s = constp.tile([1, P], bf16, tag="b1")
            nc.sync.dma_start(b1s[:], b1_t[:])
            oness = constp.tile([1, QW], bf16, tag="ones")
            nc.sync.dma_start(oness[:], ones_t[:])
            w2v = w2s[:].rearrange("p (two m) -> p two m", two=2)
            w2fr = w2fc[:]
            zt = constp.tile([P, 2 * QW], f32, tag="zt")
            nc.vector.memset(zt[:], 0.0)

            def emit_scatter(pq, par, pacc, psc, podd):
                """pa = scalar pairs (fp8 DR) + odd row + acc halves (f32r)."""
                pa = p2.tile([P, QW], f32, tag="p2")
                total = len(psc) + (1 if podd is not None else 0) + (
                    2 if pacc is not None else 0)
                k = 0
                for p in psc:
                    nc.tensor.matmul(
                        pa[:], w2v, par[:, 2 * p : 2 * p + 2, :],
                        start=(k == 0), stop=(k == total - 1),
                        perf_mode=mybir.MatmulPerfMode.DoubleRow,
                    )
                    k += 1
                if podd is not None:
                    nc.tensor.matmul(
                        pa[:], w2v[:, 0, :], par[:, podd, :],
                        start=(k == 0), stop=(k == total - 1),
                    )
                    k += 1
                if pacc is not None:
                    for h in range(2):
                        nc.tensor.matmul(
                            pa[:], w2fr,
                            pacc[:, h * QW : (h + 1) * QW],
                            start=(k == 0), stop=(k == total - 1),
                        )
                        k += 1
                ot = outp.tile([P, QW], f32, tag="ot")
                if pq % 2 == 0:
                    nc.scalar.activation(
                        ot[:], pa[:], mybir.ActivationFunctionType.Copy
                    )
                else:
                    nc.vector.tensor_copy(ot[:], pa[:])
                nc.sync.dma_start(out_t[pq], ot[:])

            pending = None
            kpair = 0
            for q in range(nquad):
                NT = int(NTq[q])
                qs = int(qstart[q])
                npair = NT // 2
                xm = xmp.tile([P, NTmax * QW], f8e3, tag="xm")
                nc.sync.dma_start(
                    xm[:, : NT * QW], xm_t[:, qs * QW : (qs + NT) * QW]
                )
                ar = arenap.tile([P, NTmax, QW], f8e4, tag="ar")
                acc = None
                psc = []
                podd = None
                for p in range(npair):
                    ph2 = p1.tile([P, 2 * QW], f32, tag="p1")
                    for h in range(2):
                        t = 2 * p + h
                        nc.tensor.matmul(
                            ph2[:, h * QW : (h + 1) * QW], w1s[:],
                            xm[:, t * QW : (t + 1) * QW],
                            start=True, stop=not has_b1,
                        )
                        if has_b1:
                            nc.tensor.matmul(
                                ph2[:, h * QW : (h + 1) * QW], b1s[:],
                                oness[:], start=False, stop=True,
                            )
                    if _is_scalar_pair(kpair):
                        psc.append(p)
                        ph2v = ph2[:].rearrange("p (two w) -> p two w", two=2)
                        nc.scalar.activation(
                            ar[:, 2 * p : 2 * p + 2, :], ph2v, relu
                        )
                    else:
                        prev = zt[:] if acc is None else acc[:]
                        if acc is None:
                            acc = accp.tile([P, 2 * QW], f32r, tag="acc")
                        nc.vector.scalar_tensor_tensor(
                            acc[:], ph2[:], 0.0, prev,
                            op_max, mybir.AluOpType.add,
                        )
                    kpair += 1
                if NT % 2 == 1:
                    t = NT - 1
                    ph2 = p1.tile([P, 2 * QW], f32, tag="p1")
                    nc.tensor.matmul(
                        ph2[:, :QW], w1s[:], xm[:, t * QW : (t + 1) * QW],
                        start=True, stop=not has_b1,
                    )
                    if has_b1:
                        nc.tensor.matmul(
                            ph2[:, :QW], b1s[:], oness[:],
                            start=False, stop=True,
                        )
                    if acc is not None:
                        nc.vector.scalar_tensor_tensor(
                            acc[:, :QW], ph2[:, :QW], 0.0, acc[:, :QW],
                            op_max, mybir.AluOpType.add,
                        )
                    else:
                        podd = t
                        nc.scalar.activation(ar[:, t, :], ph2[:, :QW], relu)
                if pending is not None:
                    emit_scatter(*pending)
                pending = (q, ar, acc, psc, podd)
            if pending is not None:
                emit_scatter(*pending)

    nc.compile()
    return nc


def kernel(x, edge_src, edge_dst, W1, b1, W2, b2, _trace=False, _ret_stats=False):
    import ml_dtypes

    x = np.asarray(x, np.float32)
    edge_src = np.asarray(edge_src).astype(np.int64)
    edge_dst = np.asarray(edge_dst).astype(np.int64)
    W1 = np.asarray(W1, np.float32).reshape(D, D)
    W2 = np.asarray(W2, np.float32).reshape(D, D)
    b1 = np.asarray(b1, np.float32).reshape(D)
    b2 = np.asarray(b2, np.float32).reshape(D)
    has_b1 = bool(np.any(b1))

    plan = _build_host_plan(x, edge_src, edge_dst)

    w1s = (16.0 * W1).astype(ml_dtypes.float8_e3m4)
    w2sc = (8.0 * W2).astype(ml_dtypes.float8_e4m3)
    w2d = np.concatenate([w2sc, w2sc], axis=1)        # [128, 2*128] both planes
    w2f = (8.0 * W2).astype(np.float32)
    b1s = (16.0 * b1).reshape(1, P).astype(ml_dtypes.bfloat16)
    ones = np.ones((1, QW), ml_dtypes.bfloat16)

    in_maps = []
    for c in range(NCORES):
        in_maps.append({
            "xm": plan["xm"][c],
            "w1": w1s,
            "w2": w2d,
            "w2f": w2f,
            "b1": b1s,
            "ones": ones,
        })

    nc = _build_program(plan, has_b1)
    res = run_bass_kernel_spmd(nc, in_maps, core_ids=list(range(NCORES)), trace=_trace)

    N = plan["N"]
    nquad = plan["nquad"]
    NTq = plan["NTq"]
    deg = plan["deg"]
    core_of = plan["core_of"]
    blk_of = plan["blk_of"]
    off_of = plan["off_of"]

    # exact de-scaling + corrections (see module docstring)
    # device pa[of, col] = sum_msgs (16*h)*(8*W2); pads contribute
    # relu(16*b1)@(8*W2) each (zero when b1 == 0).
    relu_b1 = np.maximum(16.0 * b1, 0.0)
    padvec = relu_b1 @ W2 / 16.0                      # per-pad-slot pollution/128

    out = np.zeros((N, D), np.float32)
    nodes = np.arange(N)
    q_of = blk_of[nodes] // QB
    col_of = (blk_of[nodes] % QB) * P + off_of[nodes]
    for c in range(NCORES):
        o = np.asarray(res.results[c]["out"], np.float32)   # [nquad, 128, 512]
        m = core_of == c
        nn = nodes[m]
        out[nn] = o[q_of[nn], :, col_of[nn]] / 128.0
    if has_b1:
        out -= (NTq[q_of] - deg)[:, None] * padvec[None, :]
    if np.any(b2):
        out += b2[None, :]

    if _ret_stats:
        return out, res
    return out
